# revision 33
# baseline (speedup 1.0000x reference)
"""Trainium2 Bass kernel for nn_Classifier_custom_12936441496172.

Reference math (per batch b, with av = column-l2-normalized img_b [Cf, R]):
    A      = softmax_r( (vv @ W1) @ av )          # [I, R] attention over R
    F_p    = A @ av.T                             # [I, Cf]
    out[b] = rowsum( (vv @ W2) * F_p )            # [I]

Identity used: out[b, i] = sum_r A[i, r] * ((vv @ W2) @ av)[i, r], so F_p is
never materialized. Q = vv@W1 and P = vv@W2 come from one stacked weight
matrix qpt (host-prepped; parameter-only work).

v8 final (~110us, from v6 @121us): transposed-output orientation (r on
partitions) with every measured bottleneck restructured:
- DMA: fat partition-major descriptors (host layouts [.., 128, 8KB/part]),
  ~22 total vs v6's 137 at ~650ns serial issue on the sync engine. g0's x
  ships rc-major interleaved with qpt quarters so rc0 mains stream as
  chunks land; xt rides the Scalar engine's parallel HWDGE queue; x/xt
  bufs=2 so buffer-reuse WAR waits throttle later groups' transfers out
  of the bandwidth-bound pipeline fill.
- norm-chain copy (imgt) ships as fp8e4m3: n2 sums 1024 independently
  quantized squares, so rn error is ~0.25% (overall rel_err 9.4e-3 vs the
  2e-2 gate). Full-fp8 mains were numerically rejected (6.7e-2): softmax
  logits are 1024-term dots whose per-term quant noise lands directly as
  absolute output error. DVE tensor_tensor_reduce crashes the runtime
  (micro-tested) so all four per-group squares stay on ACT accum_out.
- Mains split Q|P 312|312: symmetric PSUM banks, single STT drain per rc.
- den/num partition sums via accumulating ones-matmuls off the E/pv tiles
  (PSUM rows b0@0, b1@32), replacing v6's DVE pair-adds + deferred
  column-tiled reductions + row-gather DMAs. Issued 2-3 rc-slots after
  their inputs so the in-order PE queue never waits on ACT/DVE drains.
- division pipelined per group: DVE reciprocal of the den bank + one row
  mul + a 2-row strided out DMA while the next group's mains run; the
  last group's chain splits down the free axis to halve the tail.
- ACT queue order: Exp drains lead every slot; the next group's n2 squares
  split across the rc2/rc3 slots behind them (ACT is strict FIFO at ~65%
  load -- square latency must never gate an Exp, whose Q-bank WAR would
  stall the PE).
"""

import numpy as np

_PROGRAM = None

# Problem geometry (hardcoded per contract; kernel.py must be self-contained)
N_CORES = 8
NB = 16          # batches per core
R = 256          # H * W
CF = 1024        # feature channels
KC = CF // 128   # 8 contraction chunks
I = 312          # attributes
G = NB // 2      # groups of 2 batches
NR = 2 * R       # r-dim per group (2 batches)
RC = NR // 128   # 4 r-chunks per group
W_WARM = 96      # warmup matmuls (N=128): sized to cover the whole
                 # DMA fill (~6.5us) so HAM un-throttles during warmup
                 # and the mains run entirely at 2.4GHz, fully fed


def _build_program():
    import concourse.tile as tile
    from concourse import bacc, mybir

    F32 = mybir.dt.float32
    BF16 = mybir.dt.bfloat16
    FP8 = mybir.dt.float8e4
    MULT = mybir.AluOpType.mult
    ADD = mybir.AluOpType.add
    EXP = mybir.ActivationFunctionType.Exp
    SQUARE = mybir.ActivationFunctionType.Square

    nc = bacc.Bacc(
        "TRN2",
        target_bir_lowering=False,
        debug=False,
        enable_asserts=False,
        num_devices=N_CORES,
    )
    img = nc.dram_tensor("img", [G, 128, KC * NR], BF16, kind="ExternalInput").ap()
    imgt = nc.dram_tensor("imgt", [G, 128, RC * CF], FP8, kind="ExternalInput").ap()
    qpt = nc.dram_tensor("qpt", [128, KC * 2 * I], BF16, kind="ExternalInput").ap()
    out = nc.dram_tensor("out", [NB, I], F32, kind="ExternalOutput").ap()

    with tile.TileContext(nc) as tc, tc.tile_pool(name="sb", bufs=2) as sb, tc.tile_pool(
        name="ps", bufs=8, space="PSUM"
    ) as ps:
        ones_b = nc.const_aps.tensor(1.0, (128, 1), BF16)

        def load_x(g):
            x = sb.tile([128, KC, NR], BF16, tag="x", bufs=2, name=f"x{g}")
            nc.sync.dma_start(x[:, :, :], img[g])
            return x

        def load_xt(g, split=1, eng=None):
            # xt descriptors go on the Scalar engine's HWDGE queue: parallel
            # issue + a separate DMA ring, so the norm-chain inputs aren't
            # serialized behind the x/qpt stream on the sync queue.
            eng = eng or nc.scalar
            xt = sb.tile([128, RC * CF], FP8, tag="xt", bufs=2, name=f"xt{g}")
            step = RC // split
            for j in range(split):
                eng.dma_start(
                    xt[:, j * step * CF : (j + 1) * step * CF],
                    imgt[g][:, j * step * CF : (j + 1) * step * CF],
                )
            return xt

        # ---- head DMA. sync queue: qpt/x0 interleaved per k-pair (mains
        # stream each accumulation chunk as it lands); scalar queue: xt0
        # per-rc (squares start incrementally), xt1, xt2.
        # g0's x ships rc-major (one desc per r-chunk, all k) interleaved
        # with qpt quarters: rc0 mains hit full rate as soon as qpt lands.
        x0 = sb.tile([128, KC, NR], BF16, tag="x", bufs=2, name="x0")
        img0_3d = img[0].rearrange("p (k r) -> p k r", k=KC)
        qpt_sb = sb.tile([128, KC * 2 * I], BF16, tag="qpt", bufs=1, name="qpt_sb")
        xt_d = {0: load_xt(0, split=2)}
        for j in range(4):
            nc.sync.dma_start(
                qpt_sb[:, j * 4 * I : (j + 1) * 4 * I],
                qpt[:, j * 4 * I : (j + 1) * 4 * I],
            )
            nc.sync.dma_start(
                x0[:, :, j * 128 : (j + 1) * 128],
                img0_3d[:, :, j * 128 : (j + 1) * 128],
            )
        x_d = {0: x0}
        xt_d[1] = load_xt(1)
        x_d[1] = load_x(1)
        x_d[2] = load_x(2)
        xt_d[2] = load_xt(2)

        # Prime the single ACT table set (exp_and_others holds Exp+Square).
        prime = sb.tile([1, 16], F32, tag="prime", bufs=2, name="prime")
        nc.vector.memset(prime[:], 1.0)
        prime2 = sb.tile([1, 16], F32, tag="prime", bufs=2, name="prime2")
        nc.scalar.activation(prime2[:], prime[:], EXP)

        # PE warm-up: cold matmuls covering the first x DMA so the HAM clock
        # gate is released when the real stream begins.
        wsrc = sb.tile([128, 128], BF16, tag="warm", bufs=1, name="warmsrc")
        nc.vector.memset(wsrc[:], 0.0)
        wps = ps.tile([1, 128], F32, tag="den", bufs=1, name="warmps")
        for i in range(W_WARM):
            nc.tensor.matmul(
                wps[:], ones_b, wsrc[:], start=(i == 0), stop=(i == W_WARM - 1)
            )

        n2c_d = {}
        junk_d = {}

        def n2_squares(g, xt, rcs, scale=1.0):
            # per-r-partition squares accumulated over f for chunks `rcs`.
            # `scale` may be a [128,1] ones AP used purely to order the
            # squares after a prior group's Exp in the ACT queue (the
            # scheduler is readiness+priority driven, not program order).
            if g not in n2c_d:
                n2c_d[g] = sb.tile([128, RC], F32, tag="n2c", bufs=2, name=f"n2cg{g}")
                junk_d[g] = sb.tile([128, CF], BF16, tag="junk", bufs=2, name=f"jkg{g}")
            n2c, junk = n2c_d[g], junk_d[g]
            for rc in rcs:
                nc.scalar.activation(
                    junk[:],
                    xt[:, rc * CF : (rc + 1) * CF],
                    SQUARE,
                    scale=scale,
                    accum_out=n2c[:, rc : rc + 1],
                )

        def n2_newton(g):
            # Newton rsqrt on the 4 accumulated columns -> rn [128, RC] f32.
            n2c = n2c_d.pop(g)
            junk_d.pop(g)

            def col(nm):
                return sb.tile([128, RC], F32, tag="nw", bufs=8, name=f"{nm}g{g}")

            Y0 = 0.03125
            u1 = col("u1")
            nc.vector.tensor_scalar(u1[:], n2c[:], -0.5 * Y0 * Y0, 1.5, MULT, ADD)
            y = col("y1")
            nc.vector.tensor_scalar_mul(y[:], u1[:], Y0)
            for it in range(2):
                t = col(f"t{it}")
                nc.vector.tensor_mul(t[:], y[:], y[:])
                t2 = col(f"t2{it}")
                nc.vector.tensor_mul(t2[:], t[:], n2c[:])
                u = col(f"u{it}")
                nc.vector.tensor_scalar(u[:], t2[:], -0.5, 1.5, MULT, ADD)
                yn = col(f"y{2 + it}")
                nc.vector.tensor_mul(yn[:], y[:], u[:])
                y = yn
            return y

        def main_rc(g, x, rc):
            # Q: [128r, 312] and P: [128r, 312], accumulated over 8 k-chunks.
            q = ps.tile([128, I], F32, tag="Q", bufs=3, name=f"Qg{g}r{rc}")
            p = ps.tile([128, I], F32, tag="P", bufs=3, name=f"Pg{g}r{rc}")
            for k in range(KC):
                blk = x[:, k, rc * 128 : (rc + 1) * 128]
                nc.tensor.matmul(
                    q[:],
                    blk,
                    qpt_sb[:, k * 2 * I : k * 2 * I + I],
                    start=(k == 0),
                    stop=(k == KC - 1),
                )
                nc.tensor.matmul(
                    p[:],
                    blk,
                    qpt_sb[:, k * 2 * I + I : (k + 1) * 2 * I],
                    start=(k == 0),
                    stop=(k == KC - 1),
                )
            return q, p

        def drain_rc(g, rc, q, p, rnc):
            rcol = rnc[:, rc : rc + 1]
            E = sb.tile([128, I], BF16, tag="E", bufs=5, name=f"Eg{g}r{rc}")
            nc.scalar.activation(E[:], q[:], EXP, scale=rcol)
            pv = sb.tile([128, I], BF16, tag="pv", bufs=5, name=f"pg{g}r{rc}")
            nc.vector.scalar_tensor_tensor(
                out=pv[:], in0=E[:], scalar=rcol, in1=p[:], op0=MULT, op1=MULT
            )
            return E, pv

        den_d = {}
        num_d = {}

        def red_mms(g, bi, drains):
            # partition sums of E (den) and pv (num) for batch bi of group g,
            # accumulating the two r-chunks into PSUM row 32*bi. den MMs
            # lead so the epilogue reciprocal never waits on the num pair.
            if bi == 0:
                den_d[g] = ps.tile([33, I], F32, tag="den", bufs=1, name=f"den{g}")
                num_d[g] = ps.tile([33, I], F32, tag="num", bufs=1, name=f"num{g}")
            den, num = den_d[g], num_d[g]
            r0 = 32 * bi
            for j in range(2):
                E, pv = drains[2 * bi + j]
                nc.tensor.matmul(
                    den[r0 : r0 + 1, :], ones_b, E[:],
                    start=(j == 0), stop=(j == 1), tile_position=(0, r0),
                )
                nc.tensor.matmul(
                    num[r0 : r0 + 1, :], ones_b, pv[:],
                    start=(j == 0), stop=(j == 1), tile_position=(0, r0),
                )

        def epilogue(g, halves=1, bi=None):
            # rec = 1/den (rows 0,32 valid); fin = num * rec (one op, junk
            # rows between are free -- DVE cost is free-size only);
            # out rows via a partition-strided DMA. halves=2 splits the
            # chain down the free axis to shorten the last group's tail.
            # bi selects a single batch (used to drain the last group's
            # batch 0 while its rc2/rc3 mains still run).
            rec = sb.tile([33, I], F32, tag="rec", bufs=2, name=f"rec{g}")
            num = num_d.pop(g)
            den = den_d.pop(g)
            fin = sb.tile([33, I], F32, tag="fin", bufs=2, name=f"fin{g}")
            rows = slice(0, 33) if bi is None else slice(32 * bi, 32 * bi + 1)
            orow = (
                slice(2 * g, 2 * g + 2) if bi is None
                else slice(2 * g + bi, 2 * g + bi + 1)
            )
            osrc = fin[0:33:32] if bi is None else fin[rows]
            step = I // halves
            for j in range(halves):
                sl = slice(j * step, I if j == halves - 1 else (j + 1) * step)
                nc.vector.reciprocal(rec[rows, sl], den[rows, sl])
                nc.vector.tensor_mul(fin[rows, sl], num[rows, sl], rec[rows, sl])
                nc.sync.dma_start(out[orow, sl], osrc[:, sl])

        # group 0's chain runs up front (its squares precede all Exps).
        n2_squares(0, xt_d[0], range(RC))
        rnc_d = {0: n2_newton(0)}
        drains_d = {}

        for g in range(G):
            x = x_d.pop(g)
            drains = []
            drains_d[g] = drains
            for rc in range(RC):
                q, p = main_rc(g, x, rc)
                drains.append(drain_rc(g, rc, q, p, rnc_d[g]))
                if rc == 1:
                    # prev group's second batch: E3/pv3 landed ~1 slot ago
                    if g > 0:
                        red_mms(g - 1, 1, drains_d[g - 1])
                        drains_d.pop(g - 1)
                    if g + 2 < G:
                        if g >= 1:
                            x_d[g + 2] = load_x(g + 2)
                            xt_d[g + 2] = load_xt(g + 2, eng=nc.sync)
                if rc == 2:
                    if g + 1 < G:
                        n2_squares(g + 1, xt_d[g + 1], (0, 1))
                    if g > 0:
                        epilogue(g - 1)
                if rc == 3:
                    red_mms(g, 0, drains)
                    if g + 1 < G:
                        n2_squares(g + 1, xt_d[g + 1], (2, 3))
                        rnc_d[g + 1] = n2_newton(g + 1)
            rnc_d.pop(g)
            xt_d.pop(g)

        red_mms(G - 1, 1, drains_d[G - 1])
        epilogue(G - 1, halves=2)

    nc.compile()
    return nc


def _prepare(inputs):
    img = np.asarray(inputs["img"], np.float32)
    V = np.asarray(inputs["V"], np.float32)
    W1 = np.asarray(inputs["W1"], np.float32)
    W2 = np.asarray(inputs["W2"], np.float32)
    B, Cf, H, W = img.shape
    assert (B, Cf, H * W) == (N_CORES * NB, CF, R), img.shape

    import ml_dtypes

    vv = V.astype(np.float64)
    vv /= np.maximum(np.sqrt((vv * vv).sum(1, keepdims=True)), 1e-12)
    Q = vv @ W1.astype(np.float64)  # [I, CF]
    P = vv @ W2.astype(np.float64)
    stacked = np.concatenate([Q, P], axis=0)  # [624, CF]
    # qpt host layout [128, KC*624]: qpt_h[p, k*624 + i] = stacked[i, k*128+p]
    qpt_h = np.ascontiguousarray(
        stacked.T.reshape(KC, 128, 2 * I).transpose(1, 0, 2).reshape(128, KC * 2 * I)
    ).astype(ml_dtypes.bfloat16)

    # x host layout [C, G, 128, KC*NR]: x_h[c, g, p, k*NR + b*R + r] =
    # img[c*NB + 2g + b, k*128 + p, r]
    xb = img.reshape(N_CORES, G, 2, KC, 128, R).astype(ml_dtypes.bfloat16)
    xh = np.ascontiguousarray(
        xb.transpose(0, 1, 4, 3, 2, 5).reshape(N_CORES, G, 128, KC * NR)
    )
    # imgt host layout [C, G, 128, RC*CF]: imgt[c, g, p, rc*CF + f] =
    # img[c*NB + 2g + (rc//2), f, (rc%2)*128 + p]
    # norm-chain copy ships as fp8e4m3: n2 sums 1024 independently
    # quantized squares, so the rn error is ~0.25% (rel_err 9.4e-3 vs 2e-2).
    xt = img.reshape(N_CORES, G, 2, CF, 2, 128).astype(ml_dtypes.float8_e4m3)
    xt = np.ascontiguousarray(
        xt.transpose(0, 1, 5, 2, 4, 3).reshape(N_CORES, G, 128, RC * CF)
    )
    in_maps = [{"img": xh[c], "imgt": xt[c], "qpt": qpt_h} for c in range(N_CORES)]
    return in_maps


def run(inputs, **spmd_kwargs):
    """Run the kernel; returns (full_output [B, I], BassKernelResults)."""
    global _PROGRAM
    if _PROGRAM is None:
        _PROGRAM = _build_program()
    from concourse.bass_utils import run_bass_kernel_spmd

    in_maps = _prepare(inputs)
    res = run_bass_kernel_spmd(
        _PROGRAM, in_maps, core_ids=list(range(N_CORES)), **spmd_kwargs
    )
    out = np.concatenate(
        [np.asarray(res.results[c]["out"]) for c in range(N_CORES)], axis=0
    )
    return np.ascontiguousarray(out, np.float32), res


def kernel(**inputs) -> np.ndarray:
    return run(inputs)[0]


# revision 34
# speedup vs baseline: 1.1810x; 1.1810x over previous
"""Trainium2 Bass kernel for nn_Classifier_custom_12936441496172.

Reference math (per batch b, with av = column-l2-normalized img_b [Cf, R]):
    A      = softmax_r( (vv @ W1) @ av )          # [I, R] attention over R
    F_p    = A @ av.T                             # [I, Cf]
    out[b] = rowsum( (vv @ W2) * F_p )            # [I]

Identity used: out[b, i] = sum_r A[i, r] * ((vv @ W2) @ av)[i, r], so F_p is
never materialized. Q = vv@W1 and P = vv@W2 come from one stacked weight
matrix qpt (host-prepped; parameter-only work).

v8 final (~110us, from v6 @121us): transposed-output orientation (r on
partitions) with every measured bottleneck restructured:
- DMA: fat partition-major descriptors (host layouts [.., 128, 8KB/part]),
  ~22 total vs v6's 137 at ~650ns serial issue on the sync engine. g0's x
  ships rc-major interleaved with qpt quarters so rc0 mains stream as
  chunks land; xt rides the Scalar engine's parallel HWDGE queue; x/xt
  bufs=2 so buffer-reuse WAR waits throttle later groups' transfers out
  of the bandwidth-bound pipeline fill.
- norm-chain copy (imgt) ships as fp8e4m3: n2 sums 1024 independently
  quantized squares, so rn error is ~0.25% (overall rel_err 9.4e-3 vs the
  2e-2 gate). Full-fp8 mains were numerically rejected (6.7e-2): softmax
  logits are 1024-term dots whose per-term quant noise lands directly as
  absolute output error. DVE tensor_tensor_reduce crashes the runtime
  (micro-tested) so all four per-group squares stay on ACT accum_out.
- Mains split Q|P 312|312: symmetric PSUM banks, single STT drain per rc.
- den/num partition sums via accumulating ones-matmuls off the E/pv tiles
  (PSUM rows b0@0, b1@32), replacing v6's DVE pair-adds + deferred
  column-tiled reductions + row-gather DMAs. Issued 2-3 rc-slots after
  their inputs so the in-order PE queue never waits on ACT/DVE drains.
- division pipelined per group: DVE reciprocal of the den bank + one row
  mul + a 2-row strided out DMA while the next group's mains run; the
  last group's chain splits down the free axis to halve the tail.
- ACT queue order: Exp drains lead every slot; the next group's n2 squares
  split across the rc2/rc3 slots behind them (ACT is strict FIFO at ~65%
  load -- square latency must never gate an Exp, whose Q-bank WAR would
  stall the PE).
"""

import numpy as np

_PROGRAM = None

# Problem geometry (hardcoded per contract; kernel.py must be self-contained)
N_CORES = 8
NB = 16          # batches per core
R = 256          # H * W
CF = 1024        # feature channels
KC = CF // 128   # 8 contraction chunks
I = 312          # attributes
G = NB // 2      # groups of 2 batches
NR = 2 * R       # r-dim per group (2 batches)
RC = NR // 128   # 4 r-chunks per group
W_WARM = 24      # warmup matmuls (N=128, cover the first DMA;
                 # measured optimum -- 32/96 both slower: fillers cost
                 # ~190ns each and delay the DMA-paced early mains)


def _build_program():
    import concourse.tile as tile
    from concourse import bacc, mybir

    F32 = mybir.dt.float32
    BF16 = mybir.dt.bfloat16
    FP8 = mybir.dt.float8e4
    MULT = mybir.AluOpType.mult
    ADD = mybir.AluOpType.add
    EXP = mybir.ActivationFunctionType.Exp
    SQUARE = mybir.ActivationFunctionType.Square

    nc = bacc.Bacc(
        "TRN2",
        target_bir_lowering=False,
        debug=False,
        enable_asserts=False,
        num_devices=N_CORES,
    )
    img = nc.dram_tensor("img", [G, 128, KC * NR], BF16, kind="ExternalInput").ap()
    imgt = nc.dram_tensor("imgt", [G, 128, RC * CF], FP8, kind="ExternalInput").ap()
    qpt = nc.dram_tensor("qpt", [128, KC * 2 * I], BF16, kind="ExternalInput").ap()
    out = nc.dram_tensor("out", [NB, I], F32, kind="ExternalOutput").ap()

    with tile.TileContext(nc) as tc, tc.tile_pool(name="sb", bufs=2) as sb, tc.tile_pool(
        name="ps", bufs=8, space="PSUM"
    ) as ps:
        ones_b = nc.const_aps.tensor(1.0, (128, 1), BF16)

        def load_x(g):
            x = sb.tile([128, KC, NR], BF16, tag="x", bufs=2, name=f"x{g}")
            nc.sync.dma_start(x[:, :, :], img[g])
            return x

        def load_xt(g, split=1, eng=None):
            # xt descriptors go on the Scalar engine's HWDGE queue: parallel
            # issue + a separate DMA ring, so the norm-chain inputs aren't
            # serialized behind the x/qpt stream on the sync queue.
            eng = eng or nc.scalar
            xt = sb.tile([128, RC * CF], FP8, tag="xt", bufs=2, name=f"xt{g}")
            step = RC // split
            for j in range(split):
                eng.dma_start(
                    xt[:, j * step * CF : (j + 1) * step * CF],
                    imgt[g][:, j * step * CF : (j + 1) * step * CF],
                )
            return xt

        # ---- head DMA. sync queue: qpt/x0 interleaved per k-pair (mains
        # stream each accumulation chunk as it lands); scalar queue: xt0
        # per-rc (squares start incrementally), xt1, xt2.
        # g0's x ships rc-major (one desc per r-chunk, all k) interleaved
        # with qpt quarters: rc0 mains hit full rate as soon as qpt lands.
        x0 = sb.tile([128, KC, NR], BF16, tag="x", bufs=2, name="x0")
        img0_3d = img[0].rearrange("p (k r) -> p k r", k=KC)
        qpt_sb = sb.tile([128, KC * 2 * I], BF16, tag="qpt", bufs=1, name="qpt_sb")
        xt_d = {0: load_xt(0, split=2)}
        for j in range(4):
            nc.sync.dma_start(
                qpt_sb[:, j * 4 * I : (j + 1) * 4 * I],
                qpt[:, j * 4 * I : (j + 1) * 4 * I],
            )
            nc.sync.dma_start(
                x0[:, :, j * 128 : (j + 1) * 128],
                img0_3d[:, :, j * 128 : (j + 1) * 128],
            )
        x_d = {0: x0}
        xt_d[1] = load_xt(1)
        x_d[1] = load_x(1)
        x_d[2] = load_x(2)
        xt_d[2] = load_xt(2)

        # Prime the single ACT table set (exp_and_others holds Exp+Square).
        prime = sb.tile([1, 16], F32, tag="prime", bufs=2, name="prime")
        nc.vector.memset(prime[:], 1.0)
        prime2 = sb.tile([1, 16], F32, tag="prime", bufs=2, name="prime2")
        nc.scalar.activation(prime2[:], prime[:], EXP)

        # PE warm-up: cold matmuls covering the first x DMA so the HAM clock
        # gate is released when the real stream begins.
        wsrc = sb.tile([128, 128], BF16, tag="warm", bufs=1, name="warmsrc")
        nc.vector.memset(wsrc[:], 0.0)
        wps = ps.tile([1, 128], F32, tag="den", bufs=1, name="warmps")
        for i in range(W_WARM):
            nc.tensor.matmul(
                wps[:], ones_b, wsrc[:], start=(i == 0), stop=(i == W_WARM - 1)
            )

        n2c_d = {}
        junk_d = {}

        def n2_squares(g, xt, rcs, scale=1.0):
            # per-r-partition squares accumulated over f for chunks `rcs`.
            # `scale` may be a [128,1] ones AP used purely to order the
            # squares after a prior group's Exp in the ACT queue (the
            # scheduler is readiness+priority driven, not program order).
            if g not in n2c_d:
                n2c_d[g] = sb.tile([128, RC], F32, tag="n2c", bufs=2, name=f"n2cg{g}")
                junk_d[g] = sb.tile([128, CF], BF16, tag="junk", bufs=2, name=f"jkg{g}")
            n2c, junk = n2c_d[g], junk_d[g]
            for rc in rcs:
                nc.scalar.activation(
                    junk[:],
                    xt[:, rc * CF : (rc + 1) * CF],
                    SQUARE,
                    scale=scale,
                    accum_out=n2c[:, rc : rc + 1],
                )

        def n2_newton(g):
            # Newton rsqrt on the 4 accumulated columns -> rn [128, RC] f32.
            n2c = n2c_d.pop(g)
            junk_d.pop(g)

            def col(nm):
                return sb.tile([128, RC], F32, tag="nw", bufs=8, name=f"{nm}g{g}")

            Y0 = 0.03125
            u1 = col("u1")
            nc.vector.tensor_scalar(u1[:], n2c[:], -0.5 * Y0 * Y0, 1.5, MULT, ADD)
            y = col("y1")
            nc.vector.tensor_scalar_mul(y[:], u1[:], Y0)
            for it in range(2):
                t = col(f"t{it}")
                nc.vector.tensor_mul(t[:], y[:], y[:])
                t2 = col(f"t2{it}")
                nc.vector.tensor_mul(t2[:], t[:], n2c[:])
                u = col(f"u{it}")
                nc.vector.tensor_scalar(u[:], t2[:], -0.5, 1.5, MULT, ADD)
                yn = col(f"y{2 + it}")
                nc.vector.tensor_mul(yn[:], y[:], u[:])
                y = yn
            return y

        def main_rc(g, x, rc):
            # Q: [128r, 312] and P: [128r, 312], accumulated over 8 k-chunks.
            q = ps.tile([128, I], F32, tag="Q", bufs=3, name=f"Qg{g}r{rc}")
            p = ps.tile([128, I], F32, tag="P", bufs=3, name=f"Pg{g}r{rc}")
            for k in range(KC):
                blk = x[:, k, rc * 128 : (rc + 1) * 128]
                nc.tensor.matmul(
                    q[:],
                    blk,
                    qpt_sb[:, k * 2 * I : k * 2 * I + I],
                    start=(k == 0),
                    stop=(k == KC - 1),
                )
                nc.tensor.matmul(
                    p[:],
                    blk,
                    qpt_sb[:, k * 2 * I + I : (k + 1) * 2 * I],
                    start=(k == 0),
                    stop=(k == KC - 1),
                )
            return q, p

        def drain_rc(g, rc, q, p, rnc):
            rcol = rnc[:, rc : rc + 1]
            E = sb.tile([128, I], BF16, tag="E", bufs=5, name=f"Eg{g}r{rc}")
            nc.scalar.activation(E[:], q[:], EXP, scale=rcol)
            pv = sb.tile([128, I], BF16, tag="pv", bufs=5, name=f"pg{g}r{rc}")
            nc.vector.scalar_tensor_tensor(
                out=pv[:], in0=E[:], scalar=rcol, in1=p[:], op0=MULT, op1=MULT
            )
            return E, pv

        den_d = {}
        num_d = {}

        def red_mms(g, bi, drains):
            # partition sums of E (den) and pv (num) for batch bi of group g,
            # accumulating the two r-chunks into PSUM row 32*bi. den MMs
            # lead so the epilogue reciprocal never waits on the num pair.
            if bi == 0:
                den_d[g] = ps.tile([33, I], F32, tag="den", bufs=1, name=f"den{g}")
                num_d[g] = ps.tile([33, I], F32, tag="num", bufs=1, name=f"num{g}")
            den, num = den_d[g], num_d[g]
            r0 = 32 * bi
            for j in range(2):
                E, pv = drains[2 * bi + j]
                nc.tensor.matmul(
                    den[r0 : r0 + 1, :], ones_b, E[:],
                    start=(j == 0), stop=(j == 1), tile_position=(0, r0),
                )
                nc.tensor.matmul(
                    num[r0 : r0 + 1, :], ones_b, pv[:],
                    start=(j == 0), stop=(j == 1), tile_position=(0, r0),
                )

        def epilogue(g, halves=1, bi=None):
            # rec = 1/den (rows 0,32 valid); fin = num * rec (one op, junk
            # rows between are free -- DVE cost is free-size only);
            # out rows via a partition-strided DMA. halves=2 splits the
            # chain down the free axis to shorten the last group's tail.
            # bi selects a single batch (used to drain the last group's
            # batch 0 while its rc2/rc3 mains still run).
            rec = sb.tile([33, I], F32, tag="rec", bufs=2, name=f"rec{g}")
            num = num_d.pop(g)
            den = den_d.pop(g)
            fin = sb.tile([33, I], F32, tag="fin", bufs=2, name=f"fin{g}")
            rows = slice(0, 33) if bi is None else slice(32 * bi, 32 * bi + 1)
            orow = (
                slice(2 * g, 2 * g + 2) if bi is None
                else slice(2 * g + bi, 2 * g + bi + 1)
            )
            osrc = fin[0:33:32] if bi is None else fin[rows]
            step = I // halves
            for j in range(halves):
                sl = slice(j * step, I if j == halves - 1 else (j + 1) * step)
                nc.vector.reciprocal(rec[rows, sl], den[rows, sl])
                nc.vector.tensor_mul(fin[rows, sl], num[rows, sl], rec[rows, sl])
                nc.sync.dma_start(out[orow, sl], osrc[:, sl])

        # group 0's chain runs up front (its squares precede all Exps).
        n2_squares(0, xt_d[0], range(RC))
        rnc_d = {0: n2_newton(0)}
        drains_d = {}

        for g in range(G):
            x = x_d.pop(g)
            drains = []
            drains_d[g] = drains
            for rc in range(RC):
                q, p = main_rc(g, x, rc)
                drains.append(drain_rc(g, rc, q, p, rnc_d[g]))
                if rc == 1:
                    # prev group's second batch: E3/pv3 landed ~1 slot ago
                    if g > 0:
                        red_mms(g - 1, 1, drains_d[g - 1])
                        drains_d.pop(g - 1)
                    if g + 2 < G:
                        if g >= 1:
                            x_d[g + 2] = load_x(g + 2)
                            xt_d[g + 2] = load_xt(g + 2, eng=nc.sync)
                if rc == 2:
                    if g + 1 < G:
                        n2_squares(g + 1, xt_d[g + 1], (0, 1))
                    if g > 0:
                        epilogue(g - 1)
                if rc == 3:
                    red_mms(g, 0, drains)
                    if g + 1 < G:
                        n2_squares(g + 1, xt_d[g + 1], (2, 3))
                        rnc_d[g + 1] = n2_newton(g + 1)
            rnc_d.pop(g)
            xt_d.pop(g)

        red_mms(G - 1, 1, drains_d[G - 1])
        epilogue(G - 1, halves=2)

    nc.compile()
    return nc


def _prepare(inputs):
    img = np.asarray(inputs["img"], np.float32)
    V = np.asarray(inputs["V"], np.float32)
    W1 = np.asarray(inputs["W1"], np.float32)
    W2 = np.asarray(inputs["W2"], np.float32)
    B, Cf, H, W = img.shape
    assert (B, Cf, H * W) == (N_CORES * NB, CF, R), img.shape

    import ml_dtypes

    vv = V.astype(np.float64)
    vv /= np.maximum(np.sqrt((vv * vv).sum(1, keepdims=True)), 1e-12)
    Q = vv @ W1.astype(np.float64)  # [I, CF]
    P = vv @ W2.astype(np.float64)
    stacked = np.concatenate([Q, P], axis=0)  # [624, CF]
    # qpt host layout [128, KC*624]: qpt_h[p, k*624 + i] = stacked[i, k*128+p]
    qpt_h = np.ascontiguousarray(
        stacked.T.reshape(KC, 128, 2 * I).transpose(1, 0, 2).reshape(128, KC * 2 * I)
    ).astype(ml_dtypes.bfloat16)

    # x host layout [C, G, 128, KC*NR]: x_h[c, g, p, k*NR + b*R + r] =
    # img[c*NB + 2g + b, k*128 + p, r]
    xb = img.reshape(N_CORES, G, 2, KC, 128, R).astype(ml_dtypes.bfloat16)
    xh = np.ascontiguousarray(
        xb.transpose(0, 1, 4, 3, 2, 5).reshape(N_CORES, G, 128, KC * NR)
    )
    # imgt host layout [C, G, 128, RC*CF]: imgt[c, g, p, rc*CF + f] =
    # img[c*NB + 2g + (rc//2), f, (rc%2)*128 + p]
    # norm-chain copy ships as fp8e4m3: n2 sums 1024 independently
    # quantized squares, so the rn error is ~0.25% (rel_err 9.4e-3 vs 2e-2).
    xt = img.reshape(N_CORES, G, 2, CF, 2, 128).astype(ml_dtypes.float8_e4m3)
    xt = np.ascontiguousarray(
        xt.transpose(0, 1, 5, 2, 4, 3).reshape(N_CORES, G, 128, RC * CF)
    )
    in_maps = [{"img": xh[c], "imgt": xt[c], "qpt": qpt_h} for c in range(N_CORES)]
    return in_maps


def run(inputs, **spmd_kwargs):
    """Run the kernel; returns (full_output [B, I], BassKernelResults)."""
    global _PROGRAM
    if _PROGRAM is None:
        _PROGRAM = _build_program()
    from concourse.bass_utils import run_bass_kernel_spmd

    in_maps = _prepare(inputs)
    res = run_bass_kernel_spmd(
        _PROGRAM, in_maps, core_ids=list(range(N_CORES)), **spmd_kwargs
    )
    out = np.concatenate(
        [np.asarray(res.results[c]["out"]) for c in range(N_CORES)], axis=0
    )
    return np.ascontiguousarray(out, np.float32), res


def kernel(**inputs) -> np.ndarray:
    return run(inputs)[0]


# revision 35
# speedup vs baseline: 1.2311x; 1.0425x over previous
"""Trainium2 Bass kernel for nn_Classifier_custom_12936441496172.

Reference math (per batch b, with av = column-l2-normalized img_b [Cf, R]):
    A      = softmax_r( (vv @ W1) @ av )          # [I, R] attention over R
    F_p    = A @ av.T                             # [I, Cf]
    out[b] = rowsum( (vv @ W2) * F_p )            # [I]

Identity used: out[b, i] = sum_r A[i, r] * ((vv @ W2) @ av)[i, r], so F_p is
never materialized. Q = vv@W1 and P = vv@W2 come from one stacked weight
matrix qpt (host-prepped; parameter-only work).

v8 final (~110us, from v6 @121us): transposed-output orientation (r on
partitions) with every measured bottleneck restructured:
- DMA: fat partition-major descriptors (host layouts [.., 128, 8KB/part]),
  ~22 total vs v6's 137 at ~650ns serial issue on the sync engine. g0's x
  ships rc-major interleaved with qpt quarters so rc0 mains stream as
  chunks land; xt rides the Scalar engine's parallel HWDGE queue; x/xt
  bufs=2 so buffer-reuse WAR waits throttle later groups' transfers out
  of the bandwidth-bound pipeline fill.
- norm-chain copy (imgt) ships as fp8e4m3: n2 sums 1024 independently
  quantized squares, so rn error is ~0.25% (overall rel_err 9.4e-3 vs the
  2e-2 gate). Full-fp8 mains were numerically rejected (6.7e-2): softmax
  logits are 1024-term dots whose per-term quant noise lands directly as
  absolute output error. DVE tensor_tensor_reduce crashes the runtime
  (micro-tested) so all four per-group squares stay on ACT accum_out.
- Mains split Q|P 312|312: symmetric PSUM banks, single STT drain per rc.
- den/num partition sums via accumulating ones-matmuls off the E/pv tiles
  (PSUM rows b0@0, b1@32), replacing v6's DVE pair-adds + deferred
  column-tiled reductions + row-gather DMAs. Issued 2-3 rc-slots after
  their inputs so the in-order PE queue never waits on ACT/DVE drains.
- division pipelined per group: DVE reciprocal of the den bank + one row
  mul + a 2-row strided out DMA while the next group's mains run; the
  last group's chain splits down the free axis to halve the tail.
- ACT queue order: Exp drains lead every slot; the next group's n2 squares
  split across the rc2/rc3 slots behind them (ACT is strict FIFO at ~65%
  load -- square latency must never gate an Exp, whose Q-bank WAR would
  stall the PE).
"""

import numpy as np

_PROGRAM = None

# Problem geometry (hardcoded per contract; kernel.py must be self-contained)
N_CORES = 8
NB = 16          # batches per core
R = 256          # H * W
CF = 1024        # feature channels
KC = CF // 128   # 8 contraction chunks
I = 312          # attributes
G = NB // 2      # groups of 2 batches
NR = 2 * R       # r-dim per group (2 batches)
RC = NR // 128   # 4 r-chunks per group
W_WARM = 24      # warmup matmuls (N=128, cover the first DMA;
                 # measured optimum -- 32/96 both slower: fillers cost
                 # ~190ns each and delay the DMA-paced early mains)


def _build_program():
    import concourse.tile as tile
    from concourse import bacc, mybir

    F32 = mybir.dt.float32
    BF16 = mybir.dt.bfloat16
    FP8 = mybir.dt.float8e4
    MULT = mybir.AluOpType.mult
    ADD = mybir.AluOpType.add
    EXP = mybir.ActivationFunctionType.Exp
    SQUARE = mybir.ActivationFunctionType.Square

    nc = bacc.Bacc(
        "TRN2",
        target_bir_lowering=False,
        debug=False,
        enable_asserts=False,
        num_devices=N_CORES,
    )
    img = nc.dram_tensor("img", [G, 128, KC * NR], BF16, kind="ExternalInput").ap()
    imgt = nc.dram_tensor("imgt", [G, 128, RC * CF], FP8, kind="ExternalInput").ap()
    qpt = nc.dram_tensor("qpt", [128, KC * 2 * I], BF16, kind="ExternalInput").ap()
    out = nc.dram_tensor("out", [NB, I], F32, kind="ExternalOutput").ap()

    with tile.TileContext(nc) as tc, tc.tile_pool(name="sb", bufs=2) as sb, tc.tile_pool(
        name="ps", bufs=8, space="PSUM"
    ) as ps:
        ones_b = nc.const_aps.tensor(1.0, (128, 1), BF16)

        def load_x(g):
            x = sb.tile([128, KC, NR], BF16, tag="x", bufs=2, name=f"x{g}")
            nc.sync.dma_start(x[:, :, :], img[g])
            return x

        def load_xt(g, split=1, eng=None):
            # xt descriptors go on the Scalar engine's HWDGE queue: parallel
            # issue + a separate DMA ring, so the norm-chain inputs aren't
            # serialized behind the x/qpt stream on the sync queue.
            eng = eng or nc.scalar
            xt = sb.tile([128, RC * CF], FP8, tag="xt", bufs=2, name=f"xt{g}")
            step = RC // split
            for j in range(split):
                eng.dma_start(
                    xt[:, j * step * CF : (j + 1) * step * CF],
                    imgt[g][:, j * step * CF : (j + 1) * step * CF],
                )
            return xt

        # ---- head DMA. sync queue: qpt/x0 interleaved per k-pair (mains
        # stream each accumulation chunk as it lands); scalar queue: xt0
        # per-rc (squares start incrementally), xt1, xt2.
        # g0's x ships rc-major (one desc per r-chunk, all k) interleaved
        # with qpt quarters: rc0 mains hit full rate as soon as qpt lands.
        x0 = sb.tile([128, KC, NR], BF16, tag="x", bufs=2, name="x0")
        img0_3d = img[0].rearrange("p (k r) -> p k r", k=KC)
        qpt_sb = sb.tile([128, KC * 2 * I], BF16, tag="qpt", bufs=1, name="qpt_sb")
        xt_d = {0: load_xt(0, split=2)}
        for j in range(4):
            nc.sync.dma_start(
                qpt_sb[:, j * 4 * I : (j + 1) * 4 * I],
                qpt[:, j * 4 * I : (j + 1) * 4 * I],
            )
            nc.sync.dma_start(
                x0[:, :, j * 128 : (j + 1) * 128],
                img0_3d[:, :, j * 128 : (j + 1) * 128],
            )
        x_d = {0: x0}
        xt_d[1] = load_xt(1)
        x_d[1] = load_x(1)
        x_d[2] = load_x(2)
        xt_d[2] = load_xt(2)

        # Prime the single ACT table set (exp_and_others holds Exp+Square).
        prime = sb.tile([1, 16], F32, tag="prime", bufs=2, name="prime")
        nc.vector.memset(prime[:], 1.0)
        prime2 = sb.tile([1, 16], F32, tag="prime", bufs=2, name="prime2")
        nc.scalar.activation(prime2[:], prime[:], EXP)

        # PE warm-up: cold matmuls covering the first x DMA so the HAM clock
        # gate is released when the real stream begins.
        wsrc = sb.tile([128, 128], BF16, tag="warm", bufs=1, name="warmsrc")
        nc.vector.memset(wsrc[:], 0.0)
        wps = ps.tile([1, 128], F32, tag="den", bufs=1, name="warmps")
        for i in range(W_WARM):
            nc.tensor.matmul(
                wps[:], ones_b, wsrc[:], start=(i == 0), stop=(i == W_WARM - 1)
            )

        n2c_d = {}
        junk_d = {}

        def n2_squares(g, xt, rcs, scale=1.0):
            # per-r-partition squares accumulated over f for chunks `rcs`.
            # `scale` may be a [128,1] ones AP used purely to order the
            # squares after a prior group's Exp in the ACT queue (the
            # scheduler is readiness+priority driven, not program order).
            if g not in n2c_d:
                n2c_d[g] = sb.tile([128, RC], F32, tag="n2c", bufs=2, name=f"n2cg{g}")
                junk_d[g] = sb.tile([128, CF], BF16, tag="junk", bufs=2, name=f"jkg{g}")
            n2c, junk = n2c_d[g], junk_d[g]
            for rc in rcs:
                nc.scalar.activation(
                    junk[:],
                    xt[:, rc * CF : (rc + 1) * CF],
                    SQUARE,
                    scale=scale,
                    accum_out=n2c[:, rc : rc + 1],
                )

        def n2_newton(g, half):
            # Newton rsqrt on 2 of the 4 accumulated columns -> rn [128, 2].
            # Split per-half so Exp(rc0/rc1) never waits on the rc2/rc3
            # squares (the first Exp's scale gates the whole drain chain).
            n2a = n2c_d[g][:, 2 * half : 2 * half + 2]
            if half == 1:
                n2c_d.pop(g)
                junk_d.pop(g)

            def col(nm):
                return sb.tile([128, 2], F32, tag="nw", bufs=16, name=f"{nm}g{g}h{half}")

            Y0 = 0.03125
            u1 = col("u1")
            nc.vector.tensor_scalar(u1[:], n2a[:], -0.5 * Y0 * Y0, 1.5, MULT, ADD)
            y = col("y1")
            nc.vector.tensor_scalar_mul(y[:], u1[:], Y0)
            for it in range(2):
                t = col(f"t{it}")
                nc.vector.tensor_mul(t[:], y[:], y[:])
                t2 = col(f"t2{it}")
                nc.vector.tensor_mul(t2[:], t[:], n2a[:])
                u = col(f"u{it}")
                nc.vector.tensor_scalar(u[:], t2[:], -0.5, 1.5, MULT, ADD)
                yn = col(f"y{2 + it}")
                nc.vector.tensor_mul(yn[:], y[:], u[:])
                y = yn
            return y

        def main_rc(g, x, rc):
            # Q: [128r, 312] and P: [128r, 312], accumulated over 8 k-chunks.
            q = ps.tile([128, I], F32, tag="Q", bufs=3, name=f"Qg{g}r{rc}")
            p = ps.tile([128, I], F32, tag="P", bufs=3, name=f"Pg{g}r{rc}")
            for k in range(KC):
                blk = x[:, k, rc * 128 : (rc + 1) * 128]
                nc.tensor.matmul(
                    q[:],
                    blk,
                    qpt_sb[:, k * 2 * I : k * 2 * I + I],
                    start=(k == 0),
                    stop=(k == KC - 1),
                )
                nc.tensor.matmul(
                    p[:],
                    blk,
                    qpt_sb[:, k * 2 * I + I : (k + 1) * 2 * I],
                    start=(k == 0),
                    stop=(k == KC - 1),
                )
            return q, p

        def drain_rc(g, rc, q, p, rnc):
            rcol = rnc[rc // 2][:, rc % 2 : rc % 2 + 1]
            E = sb.tile([128, I], BF16, tag="E", bufs=5, name=f"Eg{g}r{rc}")
            nc.scalar.activation(E[:], q[:], EXP, scale=rcol)
            pv = sb.tile([128, I], BF16, tag="pv", bufs=5, name=f"pg{g}r{rc}")
            nc.vector.scalar_tensor_tensor(
                out=pv[:], in0=E[:], scalar=rcol, in1=p[:], op0=MULT, op1=MULT
            )
            return E, pv

        den_d = {}
        num_d = {}

        def red_mms(g, bi, drains):
            # partition sums of E (den) and pv (num) for batch bi of group g,
            # accumulating the two r-chunks into PSUM row 32*bi. den MMs
            # lead so the epilogue reciprocal never waits on the num pair.
            if bi == 0:
                den_d[g] = ps.tile([33, I], F32, tag="den", bufs=1, name=f"den{g}")
                num_d[g] = ps.tile([33, I], F32, tag="num", bufs=1, name=f"num{g}")
            den, num = den_d[g], num_d[g]
            r0 = 32 * bi
            for j in range(2):
                E, pv = drains[2 * bi + j]
                nc.tensor.matmul(
                    den[r0 : r0 + 1, :], ones_b, E[:],
                    start=(j == 0), stop=(j == 1), tile_position=(0, r0),
                )
                nc.tensor.matmul(
                    num[r0 : r0 + 1, :], ones_b, pv[:],
                    start=(j == 0), stop=(j == 1), tile_position=(0, r0),
                )

        def epilogue(g, halves=1, bi=None):
            # rec = 1/den (rows 0,32 valid); fin = num * rec (one op, junk
            # rows between are free -- DVE cost is free-size only);
            # out rows via a partition-strided DMA. halves=2 splits the
            # chain down the free axis to shorten the last group's tail.
            # bi selects a single batch (used to drain the last group's
            # batch 0 while its rc2/rc3 mains still run).
            rec = sb.tile([33, I], F32, tag="rec", bufs=2, name=f"rec{g}")
            num = num_d.pop(g)
            den = den_d.pop(g)
            fin = sb.tile([33, I], F32, tag="fin", bufs=2, name=f"fin{g}")
            rows = slice(0, 33) if bi is None else slice(32 * bi, 32 * bi + 1)
            orow = (
                slice(2 * g, 2 * g + 2) if bi is None
                else slice(2 * g + bi, 2 * g + bi + 1)
            )
            osrc = fin[0:33:32] if bi is None else fin[rows]
            step = I // halves
            for j in range(halves):
                sl = slice(j * step, I if j == halves - 1 else (j + 1) * step)
                nc.vector.reciprocal(rec[rows, sl], den[rows, sl])
                nc.vector.tensor_mul(fin[rows, sl], num[rows, sl], rec[rows, sl])
                nc.sync.dma_start(out[orow, sl], osrc[:, sl])

        # group 0's chain runs up front (its squares precede all Exps).
        n2_squares(0, xt_d[0], (0, 1))
        rn_a = n2_newton(0, 0)
        n2_squares(0, xt_d[0], (2, 3))
        rnc_d = {0: (rn_a, n2_newton(0, 1))}
        drains_d = {}

        for g in range(G):
            x = x_d.pop(g)
            drains = []
            drains_d[g] = drains
            for rc in range(RC):
                q, p = main_rc(g, x, rc)
                drains.append(drain_rc(g, rc, q, p, rnc_d[g]))
                if rc == 1:
                    # prev group's second batch: E3/pv3 landed ~1 slot ago
                    if g > 0:
                        red_mms(g - 1, 1, drains_d[g - 1])
                        drains_d.pop(g - 1)
                    if g + 2 < G:
                        if g >= 1:
                            x_d[g + 2] = load_x(g + 2)
                            xt_d[g + 2] = load_xt(g + 2, eng=nc.sync)
                if rc == 2:
                    if g + 1 < G:
                        n2_squares(g + 1, xt_d[g + 1], (0, 1))
                        rn_half = n2_newton(g + 1, 0)
                    if g > 0:
                        epilogue(g - 1)
                if rc == 3:
                    red_mms(g, 0, drains)
                    if g + 1 < G:
                        n2_squares(g + 1, xt_d[g + 1], (2, 3))
                        rnc_d[g + 1] = (rn_half, n2_newton(g + 1, 1))
            rnc_d.pop(g)
            xt_d.pop(g)

        red_mms(G - 1, 1, drains_d[G - 1])
        epilogue(G - 1, halves=2)

    nc.compile()
    return nc


def _prepare(inputs):
    img = np.asarray(inputs["img"], np.float32)
    V = np.asarray(inputs["V"], np.float32)
    W1 = np.asarray(inputs["W1"], np.float32)
    W2 = np.asarray(inputs["W2"], np.float32)
    B, Cf, H, W = img.shape
    assert (B, Cf, H * W) == (N_CORES * NB, CF, R), img.shape

    import ml_dtypes

    vv = V.astype(np.float64)
    vv /= np.maximum(np.sqrt((vv * vv).sum(1, keepdims=True)), 1e-12)
    Q = vv @ W1.astype(np.float64)  # [I, CF]
    P = vv @ W2.astype(np.float64)
    stacked = np.concatenate([Q, P], axis=0)  # [624, CF]
    # qpt host layout [128, KC*624]: qpt_h[p, k*624 + i] = stacked[i, k*128+p]
    qpt_h = np.ascontiguousarray(
        stacked.T.reshape(KC, 128, 2 * I).transpose(1, 0, 2).reshape(128, KC * 2 * I)
    ).astype(ml_dtypes.bfloat16)

    # x host layout [C, G, 128, KC*NR]: x_h[c, g, p, k*NR + b*R + r] =
    # img[c*NB + 2g + b, k*128 + p, r]
    xb = img.reshape(N_CORES, G, 2, KC, 128, R).astype(ml_dtypes.bfloat16)
    xh = np.ascontiguousarray(
        xb.transpose(0, 1, 4, 3, 2, 5).reshape(N_CORES, G, 128, KC * NR)
    )
    # imgt host layout [C, G, 128, RC*CF]: imgt[c, g, p, rc*CF + f] =
    # img[c*NB + 2g + (rc//2), f, (rc%2)*128 + p]
    # norm-chain copy ships as fp8e4m3: n2 sums 1024 independently
    # quantized squares, so the rn error is ~0.25% (rel_err 9.4e-3 vs 2e-2).
    xt = img.reshape(N_CORES, G, 2, CF, 2, 128).astype(ml_dtypes.float8_e4m3)
    xt = np.ascontiguousarray(
        xt.transpose(0, 1, 5, 2, 4, 3).reshape(N_CORES, G, 128, RC * CF)
    )
    in_maps = [{"img": xh[c], "imgt": xt[c], "qpt": qpt_h} for c in range(N_CORES)]
    return in_maps


def run(inputs, **spmd_kwargs):
    """Run the kernel; returns (full_output [B, I], BassKernelResults)."""
    global _PROGRAM
    if _PROGRAM is None:
        _PROGRAM = _build_program()
    from concourse.bass_utils import run_bass_kernel_spmd

    in_maps = _prepare(inputs)
    res = run_bass_kernel_spmd(
        _PROGRAM, in_maps, core_ids=list(range(N_CORES)), **spmd_kwargs
    )
    out = np.concatenate(
        [np.asarray(res.results[c]["out"]) for c in range(N_CORES)], axis=0
    )
    return np.ascontiguousarray(out, np.float32), res


def kernel(**inputs) -> np.ndarray:
    return run(inputs)[0]
